# revision 31
# baseline (speedup 1.0000x reference)
"""Trainium2 Bass kernel for nn_CIFARClassifier (8-block dense CNN, C=3).

Sharding: pure data parallel — batch 4096 split as 512 images per core
across 8 NeuronCores; the tiny weights/BN params are replicated (folded
host-side into per-block conv matrices + bias vectors).

Per-core layout: activations live in SBUF as [(c,h) partitions, (b,w) free]
with w padded by one zero column on each side (SAME-conv padding).  The h
index is placed with its low bits as the HIGH partition bits
(r32(c,h) = (h&1)*64 + ((h>>1)&1)*32 + c*8 + (h>>2)), so each 2x2 maxpool is
a free-dim max over w-pairs plus a partition-half max over h-pairs.  The
h-half realignment is done on the PE (identity matmul into PSUM) because
DMA instructions here only support a single sem-wait and DVE operands must
share a start partition.  A 3x3 conv = 3 PE matmuls (one per kernel column
kx, PSUM-accumulated, rhs shifted by kx into the padded columns); the
stationary operand is a host-built KxM matrix encoding (ic,ky)->(oc,ho)
mixing for all h rows at once.  BN folds into the matrix (scale) and an
ACT-fused relu(x+bias) (shift).  GAP(1/64) + the final 1x1 conv fold into
one [24,10] matmul whose lhsT is the data — logits land directly as
[batch, 10] for the log-softmax tail.

Sync discipline: this container's walrus accepts at most ONE sem-wait per
instruction, so the BIR is post-processed before compile — extra waits are
split into single-wait EventSemaphore instructions on the same engine
(_split_multiwait).  Performance shape: constants are packed into 5 DMAs,
x is host-pre-permuted so each sub-tile load is one contiguous DMA, the
four batch sub-tiles are emitted pairwise-interleaved (per-PSUM-chunk
yields) so one sub-tile's matmuls fill the other's dependency stalls, and
residual adds/GAP run per-chunk right behind each conv's PSUM drain.
"""

import numpy as np

EPS = 1e-5
B_TOTAL = 4096
N_CORES = 8
B_CORE = B_TOTAL // N_CORES  # 512
NB = 128                     # batch sub-tile per inner iteration
N_SUB = B_CORE // NB         # 4
P32, P16, P8 = 120, 56, 24   # used partitions (with pool-alignment holes)


def _rmap32(c, h):
    return (h & 1) * 64 + ((h >> 1) & 1) * 32 + c * 8 + (h >> 2)


def _rmap16(c, h):
    return (h & 1) * 32 + c * 8 + (h >> 1)


def _rmap8(c, h):
    return c * 8 + h


def _conv_mats(wp, rmap, R, P):
    """wp: [oc=3, ic=3, ky=3, kx=3] BN-folded weights -> [kx, K=P, M=P]."""
    mats = np.zeros((3, P, P), np.float32)
    for oc in range(3):
        for ho in range(R):
            m = rmap(oc, ho)
            for ic in range(3):
                for ky in range(3):
                    hi = ho + ky - 1
                    if 0 <= hi < R:
                        k = rmap(ic, hi)
                        mats[:, k, m] = wp[oc, ic, ky, :]
    return mats


def _bf16(a):
    import ml_dtypes
    return np.ascontiguousarray(np.asarray(a, np.float32).astype(ml_dtypes.bfloat16))


def _build_consts(ws, w9, gammas, betas, means, variances):
    ws = np.asarray(ws, np.float64)
    w9 = np.asarray(w9, np.float64)
    cm32 = np.zeros((2, 3, P32, P32), np.float32)
    cm16 = np.zeros((3, 3, P16, P16), np.float32)
    cm8 = np.zeros((3, 3, P8, P8), np.float32)
    bias32 = np.zeros((2, P32), np.float32)
    bias16 = np.zeros((3, P16), np.float32)
    bias8 = np.zeros((3, P8), np.float32)
    for blk in range(8):
        inv = np.asarray(gammas[blk], np.float64) / np.sqrt(
            np.asarray(variances[blk], np.float64) + EPS
        )
        wp = ws[blk] * inv[:, None, None, None]
        bb = np.asarray(betas[blk], np.float64) - np.asarray(means[blk], np.float64) * inv
        if blk < 2:
            cm32[blk] = _conv_mats(wp, _rmap32, 32, P32)
            for oc in range(3):
                for h in range(32):
                    bias32[blk, _rmap32(oc, h)] = bb[oc]
        elif blk < 5:
            cm16[blk - 2] = _conv_mats(wp, _rmap16, 16, P16)
            for oc in range(3):
                for h in range(16):
                    bias16[blk - 2, _rmap16(oc, h)] = bb[oc]
        else:
            cm8[blk - 5] = _conv_mats(wp, _rmap8, 8, P8)
            for oc in range(3):
                for h in range(8):
                    bias8[blk - 5, _rmap8(oc, h)] = bb[oc]
    ghead = np.zeros((P8, 10), np.float32)
    for c in range(3):
        for h in range(8):
            ghead[_rmap8(c, h), :] = w9[:, c, 1, 1] / 64.0
    # Pack all constants into 5 tensors (one DMA each — SWDGE issue cost is
    # ~2us per dma_start, so 37 small loads would cost real time).
    cmall32 = np.zeros((P32, 6 * P32), np.float32)
    for blk in range(2):
        for kx in range(3):
            i = blk * 3 + kx
            cmall32[:, i * P32:(i + 1) * P32] = cm32[blk, kx]
    cmall16 = np.zeros((P16, 9 * P16), np.float32)
    for blk in range(3):
        for kx in range(3):
            i = blk * 3 + kx
            cmall16[:, i * P16:(i + 1) * P16] = cm16[blk, kx]
    cmall8 = np.zeros((P8, 9 * P8), np.float32)
    for blk in range(3):
        for kx in range(3):
            i = blk * 3 + kx
            cmall8[:, i * P8:(i + 1) * P8] = cm8[blk, kx]
    biasall = np.zeros((P32, 8), np.float32)
    for blk in range(8):
        if blk < 2:
            biasall[:P32, blk] = bias32[blk]
        elif blk < 5:
            biasall[:P16, blk] = bias16[blk - 2]
        else:
            biasall[:P8, blk] = bias8[blk - 5]
    # identities for the PE-side h-pool realignment, pre-placed at the
    # partition base of the half they copy (lhsT/rhs share start partition)
    idall = np.zeros((P32, 90), np.float32)
    idall[64:120, 0:56] = np.eye(P16, dtype=np.float32)
    idall[32:56, 56:80] = np.eye(P8, dtype=np.float32)
    idall[0:24, 80:90] = ghead
    return {
        "cmall32": _bf16(cmall32), "cmall16": _bf16(cmall16),
        "cmall8": _bf16(cmall8), "biasall": biasall, "idall": _bf16(idall),
    }


def build_program(n_iters: int = 1):
    import concourse.bass as bass
    import concourse.tile as tile
    from concourse import mybir

    f32 = mybir.dt.float32
    bf16 = mybir.dt.bfloat16
    AFT = mybir.ActivationFunctionType
    ALU = mybir.AluOpType
    AX = mybir.AxisListType

    nc = bass.Bass()
    x_d = nc.dram_tensor("x", [N_SUB, 128, NB, 34], bf16, kind="ExternalInput")
    cm32_d = nc.dram_tensor("cmall32", [P32, 6 * P32], bf16, kind="ExternalInput")
    cm16_d = nc.dram_tensor("cmall16", [P16, 9 * P16], bf16, kind="ExternalInput")
    cm8_d = nc.dram_tensor("cmall8", [P8, 9 * P8], bf16, kind="ExternalInput")
    bias_d = nc.dram_tensor("biasall", [P32, 8], f32, kind="ExternalInput")
    id_d = nc.dram_tensor("idall", [P32, 90], bf16, kind="ExternalInput")
    out_d = nc.dram_tensor("out", [B_CORE, 10], f32, kind="ExternalOutput")

    with tile.TileContext(nc) as tc:
        with (
            tc.tile_pool(name="consts", bufs=1) as cpool,
            tc.tile_pool(name="acts", bufs=4) as apool,
            tc.tile_pool(name="xin", bufs=2) as xpool,
            tc.tile_pool(name="ps", bufs=3, space="PSUM") as pspool,
            tc.tile_pool(name="ptp", bufs=2, space="PSUM") as ptpool,
            tc.tile_pool(name="small", bufs=2) as spool,
            tc.tile_pool(name="resp", bufs=1) as rpool,
        ):
            # ---- constants: 5 packed tiles, 5 DMAs ----
            cma32 = cpool.tile([P32, 6 * P32], bf16, tag="cma32")
            nc.gpsimd.dma_start(out=cma32[:, :], in_=cm32_d[:, :])
            cma16 = cpool.tile([P16, 9 * P16], bf16, tag="cma16")
            nc.gpsimd.dma_start(out=cma16[:, :], in_=cm16_d[:, :])
            cma8 = cpool.tile([P8, 9 * P8], bf16, tag="cma8")
            nc.gpsimd.dma_start(out=cma8[:, :], in_=cm8_d[:, :])
            biasa = cpool.tile([P32, 8], f32, tag="biasa")
            nc.gpsimd.dma_start(out=biasa[:, :], in_=bias_d[:, :])
            ida = cpool.tile([P32, 90], bf16, tag="ida")
            nc.gpsimd.dma_start(out=ida[:, :], in_=id_d[:, :])

            cm_t = {}
            bias_t = {}
            for blk in range(8):
                if blk < 2:
                    for kx in range(3):
                        i = blk * 3 + kx
                        cm_t[(blk, kx)] = cma32[:, i * P32:(i + 1) * P32]
                    bias_t[blk] = biasa[0:P32, blk:blk + 1]
                elif blk < 5:
                    for kx in range(3):
                        i = (blk - 2) * 3 + kx
                        cm_t[(blk, kx)] = cma16[:, i * P16:(i + 1) * P16]
                    bias_t[blk] = biasa[0:P16, blk:blk + 1]
                else:
                    for kx in range(3):
                        i = (blk - 5) * 3 + kx
                        cm_t[(blk, kx)] = cma8[:, i * P8:(i + 1) * P8]
                    bias_t[blk] = biasa[0:P8, blk:blk + 1]
            id56_t = ida[:, 0:56]
            id24_t = ida[0:P16, 56:80]
            gh_t = ida[0:P8, 80:90]

            res_all = rpool.tile([128, N_SUB, 10], f32, tag="res_all")

            def conv_block(blk, P, R, nbc, src, dst, post=None):
                """dst[0:P, :, 1:R+1] = relu(conv(src) + bias); also zeroes
                dst's pad columns so dst can feed the next conv/sum.  PSUM
                tile holds TWO bank-halves (nbc batch each, b-major inside a
                bank) drained by ONE wide ACT.  Generator: yields after each
                PSUM tile so two sub-tiles can be emitted interleaved (fills
                PE stalls of one with the other's matmuls)."""
                nc.any.memset(dst[:, :, 0:1], 0.0)
                nc.any.memset(dst[:, :, R + 1:R + 2], 0.0)
                for j in range(NB // (2 * nbc)):
                    pt = pspool.tile([P, 2, nbc * R], f32, tag="pt")
                    for h in range(2):
                        b0 = (2 * j + h) * nbc
                        b1 = b0 + nbc
                        for kx in range(3):
                            nc.tensor.matmul(
                                pt[:, h, :], cm_t[(blk, kx)],
                                src[0:P, b0:b1, kx:kx + R],
                                start=(kx == 0), stop=(kx == 2))
                    B0, B1 = 2 * j * nbc, (2 * j + 2) * nbc
                    nc.scalar.activation(
                        out=dst[0:P, B0:B1, 1:R + 1],
                        in_=pt[:, :, :].rearrange("p h (b w) -> p (h b) w",
                                                  w=R),
                        func=AFT.Relu, bias=bias_t[blk], scale=1.0)
                    if post is not None:
                        post(B0, B1)
                    yield

            def subtile_stages(t_i):
                # ---- load x sub-tile (host pre-permuted to the exact
                # SBUF layout, holes and pad columns pre-zeroed): one
                # contiguous dependency-free DMA ----
                x1 = xpool.tile([128, NB, 34], bf16, tag="x1")
                nc.gpsimd.dma_start(out=x1[:, :, :], in_=x_d[t_i, :, :, :])
                yield

                # ---- 32x32 stage ----
                x2 = apool.tile([128, NB, 34], bf16, tag="A")
                s12 = apool.tile([128, NB, 34], bf16, tag="B")
                yield from conv_block(
                    0, P32, 32, 16, x1, x2,
                    post=lambda b0, b1: nc.vector.tensor_add(
                        s12[0:P32, b0:b1], x1[0:P32, b0:b1], x2[0:P32, b0:b1]))
                yield
                # conv1 with the whole 32->16 maxpool folded per 32-b chunk:
                # s123 add, w-pair max, PE h-realign, h-pair max all chase
                # the drain so there is no full-sub-tile pool barrier.  The
                # realign runs one chunk behind (deps already met) so the
                # in-order PE queue never stalls on it.
                x3 = apool.tile([128, NB, 34], bf16, tag="C")
                s123 = apool.tile([128, NB, 34], bf16, tag="A")
                wp = apool.tile([128, NB, 16], bf16, tag="C")
                x4 = apool.tile([128, NB, 18], bf16, tag="B")
                nc.any.memset(x4[:, :, 0:1], 0.0)
                nc.any.memset(x4[:, :, 17:18], 0.0)

                def realign32(b0, b1):
                    ptp = ptpool.tile([P16, 512], f32, tag="ptp")
                    nc.tensor.matmul(ptp[:, :], id56_t[64:120, :],
                                     wp[64:120, b0:b1, :], start=True, stop=True)
                    nc.vector.tensor_max(
                        x4[0:P16, b0:b1, 1:17], wp[0:P16, b0:b1, :],
                        ptp[:, :].rearrange("p (b w) -> p b w", w=16))

                pend2 = []

                def post2(b0, b1):
                    nc.vector.tensor_add(
                        s123[0:P32, b0:b1], s12[0:P32, b0:b1], x3[0:P32, b0:b1])
                    sv = s123[0:P32, b0:b1, 1:33].rearrange(
                        "p b (x two) -> p b x two", two=2)
                    nc.vector.tensor_max(
                        wp[0:P32, b0:b1, :], sv[:, :, :, 0], sv[:, :, :, 1])
                    if pend2:
                        realign32(*pend2.pop())
                    pend2.append((b0, b1))
                yield from conv_block(1, P32, 32, 16, s12, x3, post=post2)
                realign32(*pend2.pop())
                yield

                # ---- 16x16 stage ----
                x5 = apool.tile([128, NB, 18], bf16, tag="A")
                s45 = apool.tile([128, NB, 18], bf16, tag="E")
                yield from conv_block(
                    2, P16, 16, 32, x4, x5,
                    post=lambda b0, b1: nc.vector.tensor_add(
                        s45[0:P16, b0:b1], x4[0:P16, b0:b1], x5[0:P16, b0:b1]))
                yield
                x6 = apool.tile([128, NB, 18], bf16, tag="C")
                t56 = apool.tile([128, NB, 18], bf16, tag="F")
                s456 = apool.tile([128, NB, 18], bf16, tag="A")
                def post3(b0, b1):
                    nc.vector.tensor_add(
                        t56[0:P16, b0:b1], x5[0:P16, b0:b1], x6[0:P16, b0:b1])
                    nc.vector.tensor_add(
                        s456[0:P16, b0:b1], s45[0:P16, b0:b1], x6[0:P16, b0:b1])
                yield from conv_block(3, P16, 16, 32, s45, x6, post=post3)
                yield
                # conv4 with the 16->8 maxpool folded per 64-b chunk
                x7 = apool.tile([128, NB, 18], bf16, tag="D")
                s567 = apool.tile([128, NB, 18], bf16, tag="B")
                wp2 = apool.tile([128, NB, 8], bf16, tag="A")
                x8 = apool.tile([128, NB, 10], bf16, tag="D")
                nc.any.memset(x8[:, :, 0:1], 0.0)
                nc.any.memset(x8[:, :, 9:10], 0.0)

                def realign16(b0, b1):
                    ptp = ptpool.tile([P8, 512], f32, tag="ptp")
                    nc.tensor.matmul(ptp[:, :], id24_t[32:56, :],
                                     wp2[32:56, b0:b1, :], start=True, stop=True)
                    nc.vector.tensor_max(
                        x8[0:P8, b0:b1, 1:9], wp2[0:P8, b0:b1, :],
                        ptp[:, :].rearrange("p (b w) -> p b w", w=8))

                pend4 = []

                def post4(b0, b1):
                    nc.vector.tensor_add(
                        s567[0:P16, b0:b1], t56[0:P16, b0:b1], x7[0:P16, b0:b1])
                    sv = s567[0:P16, b0:b1, 1:17].rearrange(
                        "p b (x two) -> p b x two", two=2)
                    nc.vector.tensor_max(
                        wp2[0:P16, b0:b1, :], sv[:, :, :, 0], sv[:, :, :, 1])
                    if pend4:
                        realign16(*pend4.pop())
                    pend4.append((b0, b1))
                yield from conv_block(4, P16, 16, 32, s456, x7, post=post4)
                realign16(*pend4.pop())
                yield

                # ---- 8x8 stage ----
                x9 = apool.tile([128, NB, 10], bf16, tag="C")
                s89 = apool.tile([128, NB, 10], bf16, tag="F")
                yield from conv_block(
                    5, P8, 8, 64, x8, x9,
                    post=lambda b0, b1: nc.vector.tensor_add(
                        s89[0:P8, b0:b1], x8[0:P8, b0:b1], x9[0:P8, b0:b1]))
                yield
                x10 = apool.tile([128, NB, 10], bf16, tag="E")
                s8910 = apool.tile([128, NB, 10], bf16, tag="C")
                yield from conv_block(
                    6, P8, 8, 64, s89, x10,
                    post=lambda b0, b1: nc.vector.tensor_add(
                        s8910[0:P8, b0:b1], s89[0:P8, b0:b1], x10[0:P8, b0:b1]))
                yield
                x11 = apool.tile([128, NB, 10], bf16, tag="D")
                # ---- GAP folded into blk7's chunk loop ----
                gsum = gsums[t_i]

                def post7(b0, b1):
                    # 8-element partial sum; bf16 rounding is ~0.4% here
                    with nc.allow_low_precision(reason="bf16 GAP partials"):
                        nc.vector.reduce_sum(
                            out=gsum[:, b0:b1], in_=x11[0:P8, b0:b1, 1:9],
                            axis=AX.X)
                yield from conv_block(7, P8, 8, 64, s8910, x11, post=post7)
                yield

            def tails():
                # Softmax tails for all four sub-tiles, batched and grouped
                # by ACT function: the HW activation engine pays a table
                # reload on every function change, so Exp x4 then Ln x1
                # costs 2 switches per iteration instead of ~12.
                ph = ptpool.tile([128, N_SUB, 10], f32, tag="ptp")
                for k in range(N_SUB):
                    nc.tensor.matmul(ph[:, k, :], gsums[k], gh_t,
                                     start=True, stop=True)
                mx = spool.tile([128, N_SUB, 1], f32, tag="m")
                nc.vector.reduce_max(out=mx[:, :, :], in_=ph[:, :, :],
                                     axis=AX.X)
                negm = spool.tile([128, N_SUB], f32, tag="negm")
                nc.vector.tensor_scalar_mul(
                    negm[:, :], mx[:, :, :].rearrange("p k one -> p (k one)"),
                    -1.0)
                ex = spool.tile([128, N_SUB, 10], f32, tag="e")
                ssum = spool.tile([128, N_SUB], f32, tag="ssum")
                for k in range(N_SUB):
                    nc.scalar.activation(
                        out=ex[:, k, :], in_=ph[:, k, :], func=AFT.Exp,
                        bias=negm[:, k:k + 1], scale=1.0,
                        accum_out=ssum[:, k:k + 1])
                ls = spool.tile([128, N_SUB], f32, tag="ls")
                nc.scalar.activation(out=ls[:, :], in_=ssum[:, :], func=AFT.Ln)
                for k in range(N_SUB):
                    nc.vector.tensor_scalar(
                        out=res_all[:, k, :], in0=ph[:, k, :],
                        scalar1=negm[:, k:k + 1], scalar2=ls[:, k:k + 1],
                        op0=ALU.add, op1=ALU.subtract)

            # Two sub-tiles emitted in lockstep: their engine-queue
            # instructions interleave, so one sub-tile's matmuls fill the
            # other's dependency stalls.  n_iters>1 repeats the whole
            # forward back-to-back in one NEFF (steady-state timing).
            for _ in range(n_iters):
                gsums = [spool.tile([P8, NB], bf16, tag=f"g{k}",
                                    name=f"gsum{k}")
                         for k in range(N_SUB)]
                for a in range(0, N_SUB, 2):
                    gens = [subtile_stages(a), subtile_stages(a + 1)]
                    while gens:
                        gens = [g for g in gens if next(g, 1) is None]
                tails()

                # single output DMA (the one wait it carries is the DVE tick
                # of the last res_all write; HWDGE lane 0 is virgin)
                dst = bass.AP(tensor=out_d, offset=0,
                              ap=[[10, 128], [NB * 10, N_SUB], [1, 10]])
                nc.sync.dma_start(out=dst, in_=res_all[:, :, :])

    return nc


def _prep_x(shard):
    """[B_CORE,3,32,32] -> [N_SUB,128,NB,34] in the kernel's SBUF layout
    (h-permuted partitions, zero pool-hole rows, zero w-pad columns)."""
    xs = shard.reshape(N_SUB, NB, 3, 32, 32)
    xp = np.zeros((N_SUB, 128, NB, 34), np.float32)
    for c in range(3):
        for h in range(32):
            xp[:, _rmap32(c, h), :, 1:33] = xs[:, :, c, h, :]
    return _bf16(xp)


def _make_in_maps(x, consts):
    x = np.ascontiguousarray(np.asarray(x, np.float32))
    in_maps = []
    for i in range(N_CORES):
        shard = x[i * B_CORE:(i + 1) * B_CORE]
        m = {"x": _prep_x(shard)}
        m.update(consts)
        in_maps.append(m)
    return in_maps


_PATCHED = False


def _split_multiwait(bir_json):
    """Two BIR fixups before walrus:

    1. Dead-wait elimination: a wait on a semaphore whose threshold is
       already met by sem-inc updates of EARLIER instructions on the SAME
       engine queue is implied by the queue's in-order issue — drop it.
       DMACopy updates are excluded (their sems fire at async transfer
       completion, not issue).  Counts reset per basic block (conservative
       across branches).  Sems are monotonic, so same-queue updates alone
       reaching the threshold proves the wait satisfied.

    2. Walrus in this container accepts at most ONE sem-wait per
       instruction (setupSyncWait: 'Too many sync wait commands').  Tile's
       scheduler freely emits several.  Split the extras into single-wait
       EventSemaphore instructions on the same engine, immediately before
       the original instruction — same queue, so the sequencer performs
       the waits in order before issuing it."""
    import json
    d = json.loads(bir_json)
    cnt = 0
    for fn in d.get("functions", []):
        bkey = "basic_blocks" if "basic_blocks" in fn else "blocks"
        for blk in fn.get(bkey, []):
            counts = {}
            for inst in blk["instructions"]:
                si = inst.get("sync_info")
                eng = inst.get("engine")
                ws = (si or {}).get("on_wait") or []
                if ws:
                    keep = []
                    for w in ws:
                        if (w.get("sync_type") == "semaphore"
                                and w.get("wait_mode") == "sem-ge-imm"
                                and counts.get((eng, w.get("id")), 0)
                                >= w.get("wait_value", 1 << 62)):
                            continue
                        keep.append(w)
                    si["on_wait"] = keep
                if inst.get("opcode") != "DMACopy":
                    for u in (si or {}).get("on_update") or []:
                        if (u.get("sync_type") == "semaphore"
                                and u.get("update_mode") == "sem-inc"):
                            k = (eng, u.get("id"))
                            counts[k] = counts.get(k, 0) + u.get(
                                "update_value", 1)
            out = []
            for inst in blk["instructions"]:
                si = inst.get("sync_info")
                ws = (si or {}).get("on_wait") or []
                if len(ws) > 1:
                    for w in ws[:-1]:
                        cnt += 1
                        out.append({
                            "debug": inst.get("debug", 0),
                            "engine": inst["engine"],
                            "ins": [], "outs": [],
                            "name": f"swsplit_{cnt}",
                            "opcode": "EventSemaphore",
                            "sync_info": {"on_wait": [w], "on_update": []},
                        })
                    si["on_wait"] = [ws[-1]]
                out.append(inst)
            blk["instructions"] = out
    return json.dumps(d).encode()


def _install_compile_patch():
    global _PATCHED
    if _PATCHED:
        return
    import concourse.bass_utils as _bu
    import concourse.bass2jax as _b2j

    orig = _bu.compile_bir_kernel

    def patched(bir_json, tmpdir, neff_name="file.neff"):
        return orig(_split_multiwait(bir_json), tmpdir, neff_name)

    _bu.compile_bir_kernel = patched
    _b2j.compile_bir_kernel = patched
    _PATCHED = True


def run(x, consts, trace=False):
    from concourse.bass_utils import run_bass_kernel_spmd

    _install_compile_patch()
    nc = build_program()
    res = run_bass_kernel_spmd(
        nc, _make_in_maps(x, consts), list(range(N_CORES)), trace=trace)
    out = np.concatenate([res.results[i]["out"] for i in range(N_CORES)], axis=0)
    return out, res


def time_warm(x, consts, iters=10):
    """Time warm executions of the compiled NEFF across all 8 cores.

    Rebuilds the pjrt callable (NEFF comes from the compile cache), keeps
    inputs resident on device, and times repeated dispatches."""
    import time
    import jax
    from jax.sharding import Mesh, PartitionSpec, NamedSharding
    from jax.experimental.shard_map import shard_map
    from concourse import bass2jax, mybir

    _install_compile_patch()
    nc = build_program()
    bass2jax.install_neuronx_cc_hook()
    in_maps = _make_in_maps(x, consts)

    partition_name = (nc.partition_id_tensor.name
                      if nc.partition_id_tensor else None)
    in_names, out_names, out_avals, zero_outs = [], [], [], []
    for alloc in nc.m.functions[0].allocations:
        if not isinstance(alloc, mybir.MemoryLocationSet):
            continue
        name = alloc.memorylocations[0].name
        if alloc.kind == "ExternalInput":
            if name != partition_name:
                in_names.append(name)
        elif alloc.kind == "ExternalOutput":
            shape = tuple(alloc.tensor_shape)
            dtype = mybir.dt.np(alloc.dtype)
            out_names.append(name)
            out_avals.append(jax.core.ShapedArray(shape, dtype))
            zero_outs.append(np.zeros(shape, dtype))
    n_params = len(in_names)
    n_outs = len(out_names)
    all_names = in_names + out_names
    if partition_name is not None:
        all_names = all_names + [partition_name]
    donate = tuple(range(n_params, n_params + n_outs))

    def _body(*args):
        operands = list(args)
        if partition_name is not None:
            operands.append(bass2jax.partition_id_tensor())
        outs = bass2jax._bass_exec_p.bind(
            *operands,
            out_avals=tuple(out_avals),
            in_names=tuple(all_names),
            out_names=tuple(out_names),
            lowering_input_output_aliases=(),
            sim_require_finite=True,
            sim_require_nnan=True,
            nc=nc,
        )
        return tuple(outs)

    devices = jax.devices()[:N_CORES]
    mesh = Mesh(np.asarray(devices), ("core",))
    in_specs = (PartitionSpec("core"),) * (n_params + n_outs)
    out_specs = (PartitionSpec("core"),) * n_outs
    sharded = jax.jit(
        shard_map(_body, mesh=mesh, in_specs=in_specs, out_specs=out_specs,
                  check_rep=False),
        donate_argnums=donate, keep_unused=True)

    sh = NamedSharding(mesh, PartitionSpec("core"))
    concat_in = [
        jax.device_put(
            np.concatenate([np.asarray(in_maps[c][name]) for c in range(N_CORES)],
                           axis=0), sh)
        for name in in_names
    ]
    for a in concat_in:
        a.block_until_ready()

    def zeros():
        return [np.zeros((N_CORES * z.shape[0], *z.shape[1:]), z.dtype)
                for z in zero_outs]

    r = sharded(*concat_in, *zeros())  # warmup (compile-cache hit)
    jax.block_until_ready(r)
    # serial (includes full dispatch round-trip each call)
    best = float("inf")
    for _ in range(iters):
        zs = zeros()
        t0 = time.perf_counter()
        r = sharded(*concat_in, *zs)
        jax.block_until_ready(r)
        best = min(best, time.perf_counter() - t0)
    # pipelined back-to-back dispatches amortize the RPC round-trip
    n_pipe = 20
    zss = [zeros() for _ in range(n_pipe)]
    t0 = time.perf_counter()
    rs = [sharded(*concat_in, *z) for z in zss]
    jax.block_until_ready(rs)
    pipe = (time.perf_counter() - t0) / n_pipe
    return min(best, pipe) * 1e9


def kernel(x, ws, w9, gammas, betas, means, variances):
    consts = _build_consts(ws, w9, gammas, betas, means, variances)
    out, _ = run(x, consts, trace=False)
    return np.asarray(out, np.float32)



# revision 34
# speedup vs baseline: 1.1554x; 1.1554x over previous
"""Trainium2 Bass kernel for nn_CIFARClassifier (8-block dense CNN, C=3).

Sharding: pure data parallel — batch 4096 split as 512 images per core
across 8 NeuronCores; the tiny weights/BN params are replicated (folded
host-side into per-block conv matrices + bias vectors).

Per-core layout: activations live in SBUF as [(c,h) partitions, (b,w) free]
with w padded by one zero column on each side (SAME-conv padding).  The h
index is placed with its low bits as the HIGH partition bits
(r32(c,h) = (h&1)*64 + ((h>>1)&1)*32 + c*8 + (h>>2)), so each 2x2 maxpool is
a free-dim max over w-pairs plus a partition-half max over h-pairs.  The
h-half realignment is done on the PE (identity matmul into PSUM) because
DMA instructions here only support a single sem-wait and DVE operands must
share a start partition.  A 3x3 conv = 3 PE matmuls (one per kernel column
kx, PSUM-accumulated, rhs shifted by kx into the padded columns); the
stationary operand is a host-built KxM matrix encoding (ic,ky)->(oc,ho)
mixing for all h rows at once.  BN folds into the matrix (scale) and an
ACT-fused relu(x+bias) (shift).  GAP(1/64) + the final 1x1 conv fold into
one [24,10] matmul whose lhsT is the data — logits land directly as
[batch, 10] for the log-softmax tail.

Sync discipline: this container's walrus accepts at most ONE sem-wait per
instruction, so the BIR is post-processed before compile — extra waits are
split into single-wait EventSemaphore instructions on the same engine
(_split_multiwait).  Performance shape: constants are packed into 5 DMAs,
x is host-pre-permuted so each sub-tile load is one contiguous DMA, the
four batch sub-tiles are emitted pairwise-interleaved (per-PSUM-chunk
yields) so one sub-tile's matmuls fill the other's dependency stalls, and
residual adds/GAP run per-chunk right behind each conv's PSUM drain.
"""

import numpy as np

EPS = 1e-5
B_TOTAL = 4096
N_CORES = 8
B_CORE = B_TOTAL // N_CORES  # 512
NB = 128                     # batch sub-tile per inner iteration
N_SUB = B_CORE // NB         # 4
P32, P16, P8 = 120, 56, 24   # used partitions (with pool-alignment holes)


def _rmap32(c, h):
    return (h & 1) * 64 + ((h >> 1) & 1) * 32 + c * 8 + (h >> 2)


def _rmap16(c, h):
    return (h & 1) * 32 + c * 8 + (h >> 1)


def _rmap8(c, h):
    return c * 8 + h


def _conv_mats(wp, rmap, R, P):
    """wp: [oc=3, ic=3, ky=3, kx=3] BN-folded weights -> [kx, K=P, M=P]."""
    mats = np.zeros((3, P, P), np.float32)
    for oc in range(3):
        for ho in range(R):
            m = rmap(oc, ho)
            for ic in range(3):
                for ky in range(3):
                    hi = ho + ky - 1
                    if 0 <= hi < R:
                        k = rmap(ic, hi)
                        mats[:, k, m] = wp[oc, ic, ky, :]
    return mats


def _bf16(a):
    import ml_dtypes
    return np.ascontiguousarray(np.asarray(a, np.float32).astype(ml_dtypes.bfloat16))


def _build_consts(ws, w9, gammas, betas, means, variances):
    ws = np.asarray(ws, np.float64)
    w9 = np.asarray(w9, np.float64)
    cm32 = np.zeros((2, 3, P32, P32), np.float32)
    cm16 = np.zeros((3, 3, P16, P16), np.float32)
    cm8 = np.zeros((3, 3, P8, P8), np.float32)
    bias32 = np.zeros((2, P32), np.float32)
    bias16 = np.zeros((3, P16), np.float32)
    bias8 = np.zeros((3, P8), np.float32)
    for blk in range(8):
        inv = np.asarray(gammas[blk], np.float64) / np.sqrt(
            np.asarray(variances[blk], np.float64) + EPS
        )
        wp = ws[blk] * inv[:, None, None, None]
        bb = np.asarray(betas[blk], np.float64) - np.asarray(means[blk], np.float64) * inv
        if blk < 2:
            cm32[blk] = _conv_mats(wp, _rmap32, 32, P32)
            for oc in range(3):
                for h in range(32):
                    bias32[blk, _rmap32(oc, h)] = bb[oc]
        elif blk < 5:
            cm16[blk - 2] = _conv_mats(wp, _rmap16, 16, P16)
            for oc in range(3):
                for h in range(16):
                    bias16[blk - 2, _rmap16(oc, h)] = bb[oc]
        else:
            cm8[blk - 5] = _conv_mats(wp, _rmap8, 8, P8)
            for oc in range(3):
                for h in range(8):
                    bias8[blk - 5, _rmap8(oc, h)] = bb[oc]
    ghead = np.zeros((P8, 10), np.float32)
    for c in range(3):
        for h in range(8):
            ghead[_rmap8(c, h), :] = w9[:, c, 1, 1] / 64.0
    # Pack all constants into 5 tensors (one DMA each — SWDGE issue cost is
    # ~2us per dma_start, so 37 small loads would cost real time).
    cmall32 = np.zeros((P32, 6 * P32), np.float32)
    for blk in range(2):
        for kx in range(3):
            i = blk * 3 + kx
            cmall32[:, i * P32:(i + 1) * P32] = cm32[blk, kx]
    cmall16 = np.zeros((P16, 9 * P16), np.float32)
    for blk in range(3):
        for kx in range(3):
            i = blk * 3 + kx
            cmall16[:, i * P16:(i + 1) * P16] = cm16[blk, kx]
    cmall8 = np.zeros((P8, 9 * P8), np.float32)
    for blk in range(3):
        for kx in range(3):
            i = blk * 3 + kx
            cmall8[:, i * P8:(i + 1) * P8] = cm8[blk, kx]
    biasall = np.zeros((P32, 8), np.float32)
    for blk in range(8):
        if blk < 2:
            biasall[:P32, blk] = bias32[blk]
        elif blk < 5:
            biasall[:P16, blk] = bias16[blk - 2]
        else:
            biasall[:P8, blk] = bias8[blk - 5]
    # identities for the PE-side h-pool realignment, pre-placed at the
    # partition base of the half they copy (lhsT/rhs share start partition)
    idall = np.zeros((P32, 90), np.float32)
    idall[64:120, 0:56] = np.eye(P16, dtype=np.float32)
    idall[32:56, 56:80] = np.eye(P8, dtype=np.float32)
    idall[0:24, 80:90] = ghead
    return {
        "cmall32": _bf16(cmall32), "cmall16": _bf16(cmall16),
        "cmall8": _bf16(cmall8), "biasall": biasall, "idall": _bf16(idall),
    }


def build_program(n_iters: int = 1):
    import concourse.bass as bass
    import concourse.tile as tile
    from concourse import mybir

    f32 = mybir.dt.float32
    bf16 = mybir.dt.bfloat16
    AFT = mybir.ActivationFunctionType
    ALU = mybir.AluOpType
    AX = mybir.AxisListType

    nc = bass.Bass()
    x_d = nc.dram_tensor("x", [N_SUB, 128, NB, 34], bf16, kind="ExternalInput")
    cm32_d = nc.dram_tensor("cmall32", [P32, 6 * P32], bf16, kind="ExternalInput")
    cm16_d = nc.dram_tensor("cmall16", [P16, 9 * P16], bf16, kind="ExternalInput")
    cm8_d = nc.dram_tensor("cmall8", [P8, 9 * P8], bf16, kind="ExternalInput")
    bias_d = nc.dram_tensor("biasall", [P32, 8], f32, kind="ExternalInput")
    id_d = nc.dram_tensor("idall", [P32, 90], bf16, kind="ExternalInput")
    out_d = nc.dram_tensor("out", [B_CORE, 10], f32, kind="ExternalOutput")

    with tile.TileContext(nc) as tc:
        with (
            tc.tile_pool(name="consts", bufs=1) as cpool,
            tc.tile_pool(name="acts", bufs=4) as apool,
            tc.tile_pool(name="xin", bufs=2) as xpool,
            tc.tile_pool(name="ps", bufs=3, space="PSUM") as pspool,
            tc.tile_pool(name="ptp", bufs=2, space="PSUM") as ptpool,
            tc.tile_pool(name="small", bufs=2) as spool,
            tc.tile_pool(name="resp", bufs=1) as rpool,
        ):
            # ---- constants: 5 packed tiles, 5 DMAs ----
            cma32 = cpool.tile([P32, 6 * P32], bf16, tag="cma32")
            nc.gpsimd.dma_start(out=cma32[:, :], in_=cm32_d[:, :])
            cma16 = cpool.tile([P16, 9 * P16], bf16, tag="cma16")
            nc.gpsimd.dma_start(out=cma16[:, :], in_=cm16_d[:, :])
            cma8 = cpool.tile([P8, 9 * P8], bf16, tag="cma8")
            nc.gpsimd.dma_start(out=cma8[:, :], in_=cm8_d[:, :])
            biasa = cpool.tile([P32, 8], f32, tag="biasa")
            nc.gpsimd.dma_start(out=biasa[:, :], in_=bias_d[:, :])
            ida = cpool.tile([P32, 90], bf16, tag="ida")
            nc.gpsimd.dma_start(out=ida[:, :], in_=id_d[:, :])

            cm_t = {}
            bias_t = {}
            for blk in range(8):
                if blk < 2:
                    for kx in range(3):
                        i = blk * 3 + kx
                        cm_t[(blk, kx)] = cma32[:, i * P32:(i + 1) * P32]
                    bias_t[blk] = biasa[0:P32, blk:blk + 1]
                elif blk < 5:
                    for kx in range(3):
                        i = (blk - 2) * 3 + kx
                        cm_t[(blk, kx)] = cma16[:, i * P16:(i + 1) * P16]
                    bias_t[blk] = biasa[0:P16, blk:blk + 1]
                else:
                    for kx in range(3):
                        i = (blk - 5) * 3 + kx
                        cm_t[(blk, kx)] = cma8[:, i * P8:(i + 1) * P8]
                    bias_t[blk] = biasa[0:P8, blk:blk + 1]
            id56_t = ida[:, 0:56]
            id24_t = ida[0:P16, 56:80]
            gh_t = ida[0:P8, 80:90]

            res_all = rpool.tile([128, N_SUB, 10], f32, tag="res_all")

            def conv_block(blk, P, R, nbc, src, dst, post=None):
                """dst[0:P, :, 1:R+1] = relu(conv(src) + bias); also zeroes
                dst's pad columns so dst can feed the next conv/sum.  PSUM
                tile holds TWO bank-halves (nbc batch each, b-major inside a
                bank) drained by ONE wide ACT.  Generator: yields after each
                PSUM tile so two sub-tiles can be emitted interleaved (fills
                PE stalls of one with the other's matmuls)."""
                nc.any.memset(dst[:, :, 0:1], 0.0)
                nc.any.memset(dst[:, :, R + 1:R + 2], 0.0)
                for j in range(NB // (2 * nbc)):
                    pt = pspool.tile([P, 2, nbc * R], f32, tag="pt")
                    for h in range(2):
                        b0 = (2 * j + h) * nbc
                        b1 = b0 + nbc
                        for kx in range(3):
                            nc.tensor.matmul(
                                pt[:, h, :], cm_t[(blk, kx)],
                                src[0:P, b0:b1, kx:kx + R],
                                start=(kx == 0), stop=(kx == 2))
                    B0, B1 = 2 * j * nbc, (2 * j + 2) * nbc
                    nc.scalar.activation(
                        out=dst[0:P, B0:B1, 1:R + 1],
                        in_=pt[:, :, :].rearrange("p h (b w) -> p (h b) w",
                                                  w=R),
                        func=AFT.Relu, bias=bias_t[blk], scale=1.0)
                    if post is not None:
                        post(B0, B1)
                    yield

            def subtile_stages(t_i):
                # ---- load x sub-tile (host pre-permuted to the exact
                # SBUF layout, holes and pad columns pre-zeroed): one
                # contiguous dependency-free DMA ----
                x1 = xpool.tile([128, NB, 34], bf16, tag="x1")
                nc.gpsimd.dma_start(out=x1[:, :, :], in_=x_d[t_i, :, :, :])
                yield

                # ---- 32x32 stage ----
                x2 = apool.tile([128, NB, 34], bf16, tag="A")
                s12 = apool.tile([128, NB, 34], bf16, tag="B")
                yield from conv_block(
                    0, P32, 32, 16, x1, x2,
                    post=lambda b0, b1: nc.vector.tensor_add(
                        s12[0:P32, b0:b1], x1[0:P32, b0:b1], x2[0:P32, b0:b1]))
                yield
                # conv1 with the whole 32->16 maxpool folded per 32-b chunk:
                # s123 add, w-pair max, PE h-realign, h-pair max all chase
                # the drain so there is no full-sub-tile pool barrier.  The
                # realign runs one chunk behind (deps already met) so the
                # in-order PE queue never stalls on it.
                x3 = apool.tile([128, NB, 34], bf16, tag="C")
                s123 = apool.tile([128, NB, 34], bf16, tag="A")
                wp = apool.tile([128, NB, 16], bf16, tag="C")
                x4 = apool.tile([128, NB, 18], bf16, tag="B")
                nc.any.memset(x4[:, :, 0:1], 0.0)
                nc.any.memset(x4[:, :, 17:18], 0.0)

                def realign32(b0, b1):
                    ptp = ptpool.tile([P16, 512], f32, tag="ptp")
                    nc.tensor.matmul(ptp[:, :], id56_t[64:120, :],
                                     wp[64:120, b0:b1, :], start=True, stop=True)
                    nc.vector.tensor_max(
                        x4[0:P16, b0:b1, 1:17], wp[0:P16, b0:b1, :],
                        ptp[:, :].rearrange("p (b w) -> p b w", w=16))

                pend2 = []

                def post2(b0, b1):
                    nc.vector.tensor_add(
                        s123[0:P32, b0:b1], s12[0:P32, b0:b1], x3[0:P32, b0:b1])
                    sv = s123[0:P32, b0:b1, 1:33].rearrange(
                        "p b (x two) -> p b x two", two=2)
                    nc.vector.tensor_max(
                        wp[0:P32, b0:b1, :], sv[:, :, :, 0], sv[:, :, :, 1])
                    if pend2:
                        realign32(*pend2.pop())
                    pend2.append((b0, b1))
                yield from conv_block(1, P32, 32, 16, s12, x3, post=post2)
                realign32(*pend2.pop())
                yield

                # ---- 16x16 stage ----
                x5 = apool.tile([128, NB, 18], bf16, tag="A")
                s45 = apool.tile([128, NB, 18], bf16, tag="E")
                yield from conv_block(
                    2, P16, 16, 32, x4, x5,
                    post=lambda b0, b1: nc.vector.tensor_add(
                        s45[0:P16, b0:b1], x4[0:P16, b0:b1], x5[0:P16, b0:b1]))
                yield
                x6 = apool.tile([128, NB, 18], bf16, tag="C")
                t56 = apool.tile([128, NB, 18], bf16, tag="F")
                s456 = apool.tile([128, NB, 18], bf16, tag="A")
                def post3(b0, b1):
                    nc.vector.tensor_add(
                        t56[0:P16, b0:b1], x5[0:P16, b0:b1], x6[0:P16, b0:b1])
                    nc.vector.tensor_add(
                        s456[0:P16, b0:b1], s45[0:P16, b0:b1], x6[0:P16, b0:b1])
                yield from conv_block(3, P16, 16, 32, s45, x6, post=post3)
                yield
                # conv4 with the 16->8 maxpool folded per 64-b chunk
                x7 = apool.tile([128, NB, 18], bf16, tag="D")
                s567 = apool.tile([128, NB, 18], bf16, tag="B")
                wp2 = apool.tile([128, NB, 8], bf16, tag="A")
                x8 = apool.tile([128, NB, 10], bf16, tag="D")
                nc.any.memset(x8[:, :, 0:1], 0.0)
                nc.any.memset(x8[:, :, 9:10], 0.0)

                def realign16(b0, b1):
                    ptp = ptpool.tile([P8, 512], f32, tag="ptp")
                    nc.tensor.matmul(ptp[:, :], id24_t[32:56, :],
                                     wp2[32:56, b0:b1, :], start=True, stop=True)
                    nc.vector.tensor_max(
                        x8[0:P8, b0:b1, 1:9], wp2[0:P8, b0:b1, :],
                        ptp[:, :].rearrange("p (b w) -> p b w", w=8))

                pend4 = []

                def post4(b0, b1):
                    nc.vector.tensor_add(
                        s567[0:P16, b0:b1], t56[0:P16, b0:b1], x7[0:P16, b0:b1])
                    sv = s567[0:P16, b0:b1, 1:17].rearrange(
                        "p b (x two) -> p b x two", two=2)
                    nc.vector.tensor_max(
                        wp2[0:P16, b0:b1, :], sv[:, :, :, 0], sv[:, :, :, 1])
                    if pend4:
                        realign16(*pend4.pop())
                    pend4.append((b0, b1))
                yield from conv_block(4, P16, 16, 32, s456, x7, post=post4)
                realign16(*pend4.pop())
                yield

                # ---- 8x8 stage ----
                x9 = apool.tile([128, NB, 10], bf16, tag="C")
                s89 = apool.tile([128, NB, 10], bf16, tag="F")
                yield from conv_block(
                    5, P8, 8, 64, x8, x9,
                    post=lambda b0, b1: nc.vector.tensor_add(
                        s89[0:P8, b0:b1], x8[0:P8, b0:b1], x9[0:P8, b0:b1]))
                yield
                x10 = apool.tile([128, NB, 10], bf16, tag="E")
                s8910 = apool.tile([128, NB, 10], bf16, tag="C")
                yield from conv_block(
                    6, P8, 8, 64, s89, x10,
                    post=lambda b0, b1: nc.vector.tensor_add(
                        s8910[0:P8, b0:b1], s89[0:P8, b0:b1], x10[0:P8, b0:b1]))
                yield
                x11 = apool.tile([128, NB, 10], bf16, tag="D")
                # ---- GAP folded into blk7's chunk loop ----
                gsum = gsums[t_i]

                def post7(b0, b1):
                    # 8-element partial sum; bf16 rounding is ~0.4% here
                    with nc.allow_low_precision(reason="bf16 GAP partials"):
                        nc.vector.reduce_sum(
                            out=gsum[:, b0:b1], in_=x11[0:P8, b0:b1, 1:9],
                            axis=AX.X)
                yield from conv_block(7, P8, 8, 64, s8910, x11, post=post7)
                yield

            def tails():
                # Softmax tails for all four sub-tiles, batched and grouped
                # by ACT function: the HW activation engine pays a table
                # reload on every function change, so Exp x4 then Ln x1
                # costs 2 switches per iteration instead of ~12.
                ph = ptpool.tile([128, N_SUB, 10], f32, tag="ptp")
                for k in range(N_SUB):
                    nc.tensor.matmul(ph[:, k, :], gsums[k], gh_t,
                                     start=True, stop=True)
                mx = spool.tile([128, N_SUB, 1], f32, tag="m")
                nc.vector.reduce_max(out=mx[:, :, :], in_=ph[:, :, :],
                                     axis=AX.X)
                negm = spool.tile([128, N_SUB], f32, tag="negm")
                nc.vector.tensor_scalar_mul(
                    negm[:, :], mx[:, :, :].rearrange("p k one -> p (k one)"),
                    -1.0)
                ex = spool.tile([128, N_SUB, 10], f32, tag="e")
                ssum = spool.tile([128, N_SUB], f32, tag="ssum")
                for k in range(N_SUB):
                    nc.scalar.activation(
                        out=ex[:, k, :], in_=ph[:, k, :], func=AFT.Exp,
                        bias=negm[:, k:k + 1], scale=1.0,
                        accum_out=ssum[:, k:k + 1])
                ls = spool.tile([128, N_SUB], f32, tag="ls")
                nc.scalar.activation(out=ls[:, :], in_=ssum[:, :], func=AFT.Ln)
                for k in range(N_SUB):
                    nc.vector.tensor_scalar(
                        out=res_all[:, k, :], in0=ph[:, k, :],
                        scalar1=negm[:, k:k + 1], scalar2=ls[:, k:k + 1],
                        op0=ALU.add, op1=ALU.subtract)

            # Two sub-tiles emitted in lockstep: their engine-queue
            # instructions interleave, so one sub-tile's matmuls fill the
            # other's dependency stalls.  n_iters>1 repeats the whole
            # forward back-to-back in one NEFF (steady-state timing).
            for _ in range(n_iters):
                gsums = [spool.tile([P8, NB], bf16, tag=f"g{k}",
                                    name=f"gsum{k}")
                         for k in range(N_SUB)]
                for a in range(0, N_SUB, 2):
                    gens = [subtile_stages(a), subtile_stages(a + 1)]
                    while gens:
                        gens = [g for g in gens if next(g, 1) is None]
                tails()

                # single output DMA (the one wait it carries is the DVE tick
                # of the last res_all write; HWDGE lane 0 is virgin)
                dst = bass.AP(tensor=out_d, offset=0,
                              ap=[[10, 128], [NB * 10, N_SUB], [1, 10]])
                nc.sync.dma_start(out=dst, in_=res_all[:, :, :])

    return nc


def _prep_x(shard):
    """[B_CORE,3,32,32] -> [N_SUB,128,NB,34] in the kernel's SBUF layout
    (h-permuted partitions, zero pool-hole rows, zero w-pad columns)."""
    xs = shard.reshape(N_SUB, NB, 3, 32, 32)
    xp = np.zeros((N_SUB, 128, NB, 34), np.float32)
    for c in range(3):
        for h in range(32):
            xp[:, _rmap32(c, h), :, 1:33] = xs[:, :, c, h, :]
    return _bf16(xp)


def _make_in_maps(x, consts):
    x = np.ascontiguousarray(np.asarray(x, np.float32))
    in_maps = []
    for i in range(N_CORES):
        shard = x[i * B_CORE:(i + 1) * B_CORE]
        m = {"x": _prep_x(shard)}
        m.update(consts)
        in_maps.append(m)
    return in_maps


_PATCHED = False


def _split_multiwait(bir_json):
    """Two BIR fixups before walrus:

    1. Dead-wait elimination: a wait on a semaphore whose threshold is
       already met by sem-inc updates of EARLIER instructions on the SAME
       engine queue is implied by the queue's in-order issue — drop it.
       DMACopy updates are excluded (their sems fire at async transfer
       completion, not issue).  Counts reset per basic block (conservative
       across branches).  Sems are monotonic, so same-queue updates alone
       reaching the threshold proves the wait satisfied.

    2. Walrus in this container accepts at most ONE sem-wait per
       instruction (setupSyncWait: 'Too many sync wait commands').  Tile's
       scheduler freely emits several.  Split the extras into single-wait
       EventSemaphore instructions on the same engine, immediately before
       the original instruction — same queue, so the sequencer performs
       the waits in order before issuing it."""
    import json
    d = json.loads(bir_json)
    cnt = 0
    for fn in d.get("functions", []):
        bkey = "basic_blocks" if "basic_blocks" in fn else "blocks"
        for blk in fn.get(bkey, []):
            counts = {}
            for inst in blk["instructions"]:
                si = inst.get("sync_info")
                eng = inst.get("engine")
                ws = (si or {}).get("on_wait") or []
                if ws:
                    keep = []
                    for w in ws:
                        if (w.get("sync_type") == "semaphore"
                                and w.get("wait_mode") == "sem-ge-imm"
                                and counts.get((eng, w.get("id")), 0)
                                >= w.get("wait_value", 1 << 62)):
                            continue
                        keep.append(w)
                    si["on_wait"] = keep
                if inst.get("opcode") != "DMACopy":
                    for u in (si or {}).get("on_update") or []:
                        if (u.get("sync_type") == "semaphore"
                                and u.get("update_mode") == "sem-inc"):
                            k = (eng, u.get("id"))
                            counts[k] = counts.get(k, 0) + u.get(
                                "update_value", 1)
            out = []
            for inst in blk["instructions"]:
                si = inst.get("sync_info")
                ws = (si or {}).get("on_wait") or []
                if len(ws) > 1:
                    for w in ws[:-1]:
                        cnt += 1
                        out.append({
                            "debug": inst.get("debug", 0),
                            "engine": inst["engine"],
                            "ins": [], "outs": [],
                            "name": f"swsplit_{cnt}",
                            "opcode": "EventSemaphore",
                            "sync_info": {"on_wait": [w], "on_update": []},
                        })
                    si["on_wait"] = [ws[-1]]
                out.append(inst)
            blk["instructions"] = out
    return json.dumps(d).encode()


def _install_compile_patch():
    global _PATCHED
    if _PATCHED:
        return
    import concourse.bass_utils as _bu
    import concourse.bass2jax as _b2j

    orig = _bu.compile_bir_kernel

    def patched(bir_json, tmpdir, neff_name="file.neff"):
        return orig(_split_multiwait(bir_json), tmpdir, neff_name)

    _bu.compile_bir_kernel = patched
    _b2j.compile_bir_kernel = patched
    _PATCHED = True


def run(x, consts, trace=False):
    from concourse.bass_utils import run_bass_kernel_spmd

    _install_compile_patch()
    nc = build_program()
    res = run_bass_kernel_spmd(
        nc, _make_in_maps(x, consts), list(range(N_CORES)), trace=trace)
    out = np.concatenate([res.results[i]["out"] for i in range(N_CORES)], axis=0)
    return out, res


def time_warm(x, consts, iters=10):
    """Time warm executions of the compiled NEFF across all 8 cores.

    Rebuilds the pjrt callable (NEFF comes from the compile cache), keeps
    inputs resident on device, and times repeated dispatches."""
    import time
    import jax
    from jax.sharding import Mesh, PartitionSpec, NamedSharding
    from jax.experimental.shard_map import shard_map
    from concourse import bass2jax, mybir

    _install_compile_patch()
    nc = build_program()
    bass2jax.install_neuronx_cc_hook()
    in_maps = _make_in_maps(x, consts)

    partition_name = (nc.partition_id_tensor.name
                      if nc.partition_id_tensor else None)
    in_names, out_names, out_avals, zero_outs = [], [], [], []
    for alloc in nc.m.functions[0].allocations:
        if not isinstance(alloc, mybir.MemoryLocationSet):
            continue
        name = alloc.memorylocations[0].name
        if alloc.kind == "ExternalInput":
            if name != partition_name:
                in_names.append(name)
        elif alloc.kind == "ExternalOutput":
            shape = tuple(alloc.tensor_shape)
            dtype = mybir.dt.np(alloc.dtype)
            out_names.append(name)
            out_avals.append(jax.core.ShapedArray(shape, dtype))
            zero_outs.append(np.zeros(shape, dtype))
    n_params = len(in_names)
    n_outs = len(out_names)
    all_names = in_names + out_names
    if partition_name is not None:
        all_names = all_names + [partition_name]
    donate = tuple(range(n_params, n_params + n_outs))

    def _body(*args):
        operands = list(args)
        if partition_name is not None:
            operands.append(bass2jax.partition_id_tensor())
        outs = bass2jax._bass_exec_p.bind(
            *operands,
            out_avals=tuple(out_avals),
            in_names=tuple(all_names),
            out_names=tuple(out_names),
            lowering_input_output_aliases=(),
            sim_require_finite=True,
            sim_require_nnan=True,
            nc=nc,
        )
        return tuple(outs)

    devices = jax.devices()[:N_CORES]
    mesh = Mesh(np.asarray(devices), ("core",))
    in_specs = (PartitionSpec("core"),) * (n_params + n_outs)
    out_specs = (PartitionSpec("core"),) * n_outs
    sharded = jax.jit(
        shard_map(_body, mesh=mesh, in_specs=in_specs, out_specs=out_specs,
                  check_rep=False),
        donate_argnums=donate, keep_unused=True)

    sh = NamedSharding(mesh, PartitionSpec("core"))
    concat_in = [
        jax.device_put(
            np.concatenate([np.asarray(in_maps[c][name]) for c in range(N_CORES)],
                           axis=0), sh)
        for name in in_names
    ]
    for a in concat_in:
        a.block_until_ready()

    def zeros():
        return [np.zeros((N_CORES * z.shape[0], *z.shape[1:]), z.dtype)
                for z in zero_outs]

    r = sharded(*concat_in, *zeros())  # warmup (compile-cache hit)
    jax.block_until_ready(r)
    # serial (includes full dispatch round-trip each call)
    best = float("inf")
    for _ in range(iters):
        zs = zeros()
        t0 = time.perf_counter()
        r = sharded(*concat_in, *zs)
        jax.block_until_ready(r)
        best = min(best, time.perf_counter() - t0)
    # pipelined back-to-back dispatches amortize the RPC round-trip
    n_pipe = 20
    zss = [zeros() for _ in range(n_pipe)]
    t0 = time.perf_counter()
    rs = [sharded(*concat_in, *z) for z in zss]
    jax.block_until_ready(rs)
    pipe = (time.perf_counter() - t0) / n_pipe
    return min(best, pipe) * 1e9


def kernel(x, ws, w9, gammas, betas, means, variances):
    consts = _build_consts(ws, w9, gammas, betas, means, variances)
    out, _ = run(x, consts, trace=False)
    return np.asarray(out, np.float32)

